# revision 2
# baseline (speedup 1.0000x reference)
"""Trainium2 Bass kernel for nn_CustomParameterTransform (scatter_memory).

Reference semantics: coord_v [256, 30] holds 10 (x, y, mass) triplets per
sample. Each triplet maps to integer grid indices (x_i, y_i, m_i); a one-hot
volume z [B, 16, 128, 128] is scattered (z[b, m, y, x] = 1) and the output is
concat(1-z, z) over the channel axis -> [256, 32, 128, 128] f32 (512 MB).

Strategy (8 NeuronCores, batch-sharded, no cross-core comm):
  - The output is almost entirely constant: the first 16 channels are 1.0
    except at scatter points, the last 16 are 0.0 except at scatter points.
  - Per core (32 samples, 64 MB slab): fill the slab from constant SBUF
    tiles with large DMAs (write-only HBM traffic; ~425 GB/s sustained =
    16 SDMA engines x ~26.6 GB/s), then fix up the 640 scatter points with
    indirect (scatter) DMAs on the gpsimd/SWDGE queue.
  - Indices are computed on the host with the exact same jax ops as the
    reference (bit-identical floor/log10 behavior) and passed per-core as a
    [128, 6] int32 tensor of flat element offsets.

Trace-driven tuning (what each piece buys):
  - First memsets on the vector engine (gpsimd takes ~7 us to wake after
    the NEFF startup barrier; vector is ready at ~4.8 us) -> first fill
    at ~5.6 us instead of 8.4 us.
  - Fills run in ascending sample order; scatter columns for samples 0-29
    depend on fills that complete mid-stream, so those scatters fully
    overlap the fill phase. Only the last fill (samples 30-31) gates a
    scatter: a small 64-row column (fast SWDGE dispatch) -> post-fill tail
    ~2.5 us instead of ~4 us for a 128-row column gated on everything.
  - Light drain/barrier epilogue (see _light_drain_and_barrier). The
    remaining exit cost (~7 us: all-engine barrier + a NEFF-level sweep
    zeroing all 253 event semaphores + exit barrier) is emitted by the
    NEFF toolchain, not bass, and is not reachable from kernel code.
"""

import numpy as np

B = 256
NSRC = 10
NMC = 16
L = 128
NCORES = 8
BL = B // NCORES          # 32 samples per core
PLANE = L * L             # 16384
HALF = NMC * PLANE        # 262144 elements per half-slab
SLAB = 2 * HALF           # 524288 elements per sample
OUT_ELEMS = BL * SLAB     # 16777216 per core (64 MB)

N_SCATTER_COLS = 6        # cols 0-4: samples 0-29; col 5: samples 30-31

_CACHE = {}


def _build_nc():
    import concourse.bass as bass
    import concourse.tile as tile
    from concourse import bacc, mybir
    from concourse.tile_rust import add_dep_helper

    import types as _types
    from concourse.vector_clock import ScopedClock

    nc = bacc.Bacc("TRN2", target_bir_lowering=False, debug=False,
                   num_devices=NCORES)

    def _light_drain_and_barrier(self, tick_clock, wait_clock):
        """Replaces TileContext._drain_and_barrier for this kernel. The
        stock epilogue is drain + two all-engine EVSEM butterfly barriers
        around the sem clear. Requirements at kernel end are: (1) all DMA
        completions observed, (2) sems cleared for NEFF re-execution,
        (3) the clear happens after every engine's last sem use. (1) is
        the sync drain's global-clock waits; (3) is a counting-sem join
        (sync arrives only after the drain, so join>=4 implies all DMA
        done); (2) is the ranged clear. The second barrier is
        unnecessary: a re-execution cannot start until every engine --
        including the clearing gpsimd -- has ended."""
        nc_ = self.nc
        drain_inst = nc_.sync.drain()
        wait_clock.add_sem_waits(
            drain_inst.ins, ScopedClock({None: tick_clock.global_clock}))
        join = nc_.alloc_semaphore("tail_join")
        for eng in nc_.engines.values():
            if eng is not nc_.gpsimd:
                eng.sem_inc(join, 1)
        n_other = len(nc_.engines) - 1
        nc_.gpsimd.wait_ge(join, n_other)
        popped = nc_._tile_sem_poison_stack.pop()
        assert popped is self._sem_poison
        sems = list(self.sems.allocated().values())
        nc_.clear_and_free_semaphores(sems + [join])

    offs = nc.dram_tensor("offs", [128, N_SCATTER_COLS], mybir.dt.int32,
                          kind="ExternalInput").ap()
    out = nc.dram_tensor("out", [OUT_ELEMS], mybir.dt.float32,
                         kind="ExternalOutput").ap()

    with tile.TileContext(nc) as tc:
        tc._drain_and_barrier = _types.MethodType(_light_drain_and_barrier, tc)
        with tc.tile_pool(name="src", bufs=1) as src_pool, \
             tc.tile_pool(name="small", bufs=1) as small_pool:
            # Constant source tiles. Memset cost scales with the free-dim
            # cols (128 lanes run in parallel), so big tiles are split
            # column-wise between vector and gpsimd. Vector handles
            # everything needed early (it wakes ~2 us before gpsimd).
            ones_mini = src_pool.tile([128, 1024], mybir.dt.float32)
            zeros_mini = src_pool.tile([128, 1024], mybir.dt.float32)
            nc.vector.memset(ones_mini[:, :], 1.0)
            nc.vector.memset(zeros_mini[:, :], 0.0)
            # combo: one full slab ([128, 4096]; DMA iterates partition-
            # major, so partitions 0-63 are the ones half, 64-127 zeros).
            combo_t = src_pool.tile([128, 4096], mybir.dt.float32)
            nc.vector.memset(combo_t[0:64, 0:2048], 1.0)
            nc.vector.memset(combo_t[64:128, 0:2048], 0.0)
            nc.gpsimd.memset(combo_t[0:64, 2048:4096], 1.0)
            nc.gpsimd.memset(combo_t[64:128, 2048:4096], 0.0)
            # mega: two slabs ([128, 8192]; slab = 64 partitions, ones iff
            # p%64 < 32). Feeds samples 6-29 as 4 MB fills.
            mega_t = src_pool.tile([128, 8192], mybir.dt.float32)
            for lo, hi, v in ((0, 32, 1.0), (32, 64, 0.0),
                              (64, 96, 1.0), (96, 128, 0.0)):
                nc.vector.memset(mega_t[lo:hi, 0:4096], v)
                nc.gpsimd.memset(mega_t[lo:hi, 4096:8192], v)

            # Scatter offsets: [128, 6] int32 flat element indices.
            # Column layout (points p = 10*s + k, in order):
            #   col 0: ones-half offsets, points   0..127 (samples  0-12)
            #   col 1: z-half    offsets, points   0..127
            #   col 2: ones-half offsets, points 128..255 (samples 12-25)
            #   col 3: z-half    offsets, points 128..255
            #   col 4: rows 0-63 ones-half pts 256..299 (+dups),
            #          rows 64-127 z-half pts 256..299 (+dups)
            #   col 5 (rows 0-63 used): rows 0-31 ones-half pts 300..319
            #          (+dups), rows 32-63 z-half pts 300..319 (+dups)
            # offs loads on the scalar HWDGE queue after its first two
            # fills; vals memsets ride on vector after its big memsets.
            offs_t = small_pool.tile([128, N_SCATTER_COLS], mybir.dt.int32)
            vals_t = small_pool.tile([128, N_SCATTER_COLS], mybir.dt.float32)
            nc.vector.memset(vals_t[:, 0:1], 0.0)
            nc.vector.memset(vals_t[:, 1:2], 1.0)
            nc.vector.memset(vals_t[:, 2:3], 0.0)
            nc.vector.memset(vals_t[:, 3:4], 1.0)
            nc.vector.memset(vals_t[0:64, 4:5], 0.0)
            nc.vector.memset(vals_t[64:128, 4:5], 1.0)
            nc.vector.memset(vals_t[0:32, 5:6], 0.0)
            nc.vector.memset(vals_t[32:64, 5:6], 1.0)

            MINI = 131072  # elements per mini fill (512 KB)
            ones_fills = {}   # sample -> list of fills covering its ones half
            zeros_fills = {}  # sample -> list of fills covering its zeros half
            fill_seq = {"sync": 0, "scalar": 0}

            # Samples 0-1 from the minis (ready first).
            for s in (0, 1):
                e_ones = nc.sync if s == 0 else nc.scalar
                e_zeros = nc.scalar if s == 0 else nc.sync
                ones_fills[s] = [
                    e_ones.dma_start(
                        out[s * SLAB + k * MINI:s * SLAB + (k + 1) * MINI],
                        ones_mini[:, :])
                    for k in range(2)]
                zeros_fills[s] = [
                    e_zeros.dma_start(
                        out[s * SLAB + HALF + k * MINI:
                            s * SLAB + HALF + (k + 1) * MINI],
                        zeros_mini[:, :])
                    for k in range(2)]
            # offs load sits on the scalar queue here: after the first
            # mini fills (so it doesn't delay the stream start), done by
            # ~8 us, way before the first scatter needs it (~70 us).
            offs_fill = nc.scalar.dma_start(offs_t[:, :], offs[:, :])
            # Samples 2-5 from combo (2 MB fills).
            for s in range(2, 6):
                eng = nc.sync if s % 2 == 0 else nc.scalar
                f = eng.dma_start(out[s * SLAB:(s + 1) * SLAB], combo_t[:, :])
                ones_fills[s] = [f]
                zeros_fills[s] = [f]
            # Samples 6-29 from mega (4 MB pair fills), ascending, so the
            # scatter columns' fill deps complete early in the stream.
            for s in range(6, 30, 2):
                eng = nc.sync if (s // 2) % 2 == 0 else nc.scalar
                f = eng.dma_start(out[s * SLAB:(s + 2) * SLAB], mega_t[:, :])
                for ss in (s, s + 1):
                    ones_fills[ss] = [f]
                    zeros_fills[ss] = [f]
            # Samples 30-31 last, one 2 MB fill per queue (balances both
            # queues at 32 MB) from mega's two slab halves.
            f30 = nc.sync.dma_start(out[30 * SLAB:31 * SLAB], mega_t[0:64, :])
            f31 = nc.scalar.dma_start(out[31 * SLAB:32 * SLAB],
                                      mega_t[64:128, :])
            ones_fills[30] = [f30]
            zeros_fills[30] = [f30]
            ones_fills[31] = [f31]
            zeros_fills[31] = [f31]

            # Which sample-fills each scatter column touches.
            def deps(table, lo, hi):
                return [f for s in range(lo, hi) for f in table[s]]
            col_specs = [
                (slice(0, 128), deps(ones_fills, 0, 13)),
                (slice(0, 128), deps(zeros_fills, 0, 13)),
                (slice(0, 128), deps(ones_fills, 12, 26)),
                (slice(0, 128), deps(zeros_fills, 12, 26)),
                (slice(0, 128),
                 deps(ones_fills, 25, 30) + deps(zeros_fills, 25, 30)),
                (slice(0, 64),
                 deps(ones_fills, 30, 32) + deps(zeros_fills, 30, 32)),
            ]

            # Narrow declared out AP ([1, 1] at offset 0): the real write
            # addresses come from the offset tensor; a full-tensor AP would
            # make Tile serialize every scatter behind every fill (WAW), and
            # the explicit col_deps edges below provide the true ordering.
            out2d = out[0:1].unsqueeze(1)
            for j, (rows, fl_deps) in enumerate(col_specs):
                sc = nc.gpsimd.indirect_dma_start(
                    out=out2d,
                    out_offset=bass.IndirectOffsetOnAxis(
                        ap=offs_t[rows, j:j + 1], axis=0),
                    in_=vals_t[rows, j:j + 1],
                    in_offset=None,
                )
                for fl in fl_deps:
                    add_dep_helper(sc.ins, fl.ins,
                                   reason="scatter after its sample fills")

    nc.compile()
    return nc


def _compute_indices(coord_v, lows, highs, nmc, L_):
    """Replicates reference.py lines exactly (same jax ops on the default
    device) so the floor/log10 bin boundaries match bit-for-bit."""
    import jax.numpy as jnp

    cv = jnp.asarray(np.asarray(coord_v, dtype=np.float32))
    n = cv.shape[1] // 3
    v10 = cv.at[:, 2::3].set(jnp.log10(cv[:, 2::3]))
    lo = jnp.tile(jnp.asarray(np.asarray(lows, dtype=np.float32)), n)
    hi = jnp.tile(jnp.asarray(np.asarray(highs, dtype=np.float32)), n)
    coord_grid = (v10 - lo) / (hi - lo)
    tr = coord_grid.reshape(-1, 3)
    x_i = jnp.floor(tr[:, 0] * L_).astype(jnp.int32)
    y_i = jnp.floor(tr[:, 1] * L_).astype(jnp.int32)
    m_i = jnp.floor(tr[:, 2] * nmc).astype(jnp.int32)
    return (np.asarray(x_i), np.asarray(y_i), np.asarray(m_i))


def _prepare_in_maps(coord_v, lows, highs, nmc, L):
    nmc = int(nmc)
    L_ = int(L)
    x_i, y_i, m_i = _compute_indices(coord_v, lows, highs, nmc, L_)
    n_batch = coord_v.shape[0]
    n = coord_v.shape[1] // 3
    b_i = np.repeat(np.arange(n_batch, dtype=np.int64), n)

    # Flat element offsets (per core, local slab coordinates).
    flat_ones = ((b_i % BL) * SLAB + m_i.astype(np.int64) * PLANE
                 + y_i.astype(np.int64) * L_ + x_i.astype(np.int64))
    flat_z = flat_ones + HALF

    in_maps = []
    pts_per_core = BL * n  # 320
    for c in range(NCORES):
        sel = slice(c * pts_per_core, (c + 1) * pts_per_core)
        po = flat_ones[sel]
        pz = flat_z[sel]
        offs_np = np.zeros((128, N_SCATTER_COLS), dtype=np.int32)
        offs_np[:, 0] = po[0:128]
        offs_np[:, 1] = pz[0:128]
        offs_np[:, 2] = po[128:256]
        offs_np[:, 3] = pz[128:256]
        # col 4: samples 25-29 (44 points), padded with duplicates
        o4 = np.full(64, po[256], dtype=np.int32)
        o4[0:44] = po[256:300]
        z4 = np.full(64, pz[256], dtype=np.int32)
        z4[0:44] = pz[256:300]
        offs_np[0:64, 4] = o4
        offs_np[64:128, 4] = z4
        # col 5 (rows 0-63 used): samples 30-31 (20 points), padded
        o5 = np.full(32, po[300], dtype=np.int32)
        o5[0:20] = po[300:320]
        z5 = np.full(32, pz[300], dtype=np.int32)
        z5[0:20] = pz[300:320]
        offs_np[0:32, 5] = o5
        offs_np[32:64, 5] = z5
        in_maps.append({"offs": offs_np})
    return in_maps


def _run(in_maps, **kwargs):
    if "nc" not in _CACHE:
        _CACHE["nc"] = _build_nc()
    nc = _CACHE["nc"]
    from concourse.bass_utils import run_bass_kernel_spmd
    return run_bass_kernel_spmd(nc, in_maps, core_ids=list(range(NCORES)),
                                **kwargs)


def kernel(coord_v, lows, highs, nmc, L):
    nmc = int(nmc)
    L_ = int(L)
    assert nmc == NMC and L_ == globals()["L"], (nmc, L_)

    in_maps = _prepare_in_maps(coord_v, lows, highs, nmc, L_)
    res = _run(in_maps)
    parts = [res.results[c]["out"].reshape(BL, 2 * NMC, L_, L_)
             for c in range(NCORES)]
    return np.concatenate(parts, axis=0)
